# revision 9
# baseline (speedup 1.0000x reference)
"""Trainium2 Bass kernel for nn_Boundary_Enchance (dense_cnn).

Data-parallel over batch: core i of 8 processes image i.

Per-core pipeline (bf16 compute, fp32 PSUM accumulate):
  phase A (per 8-row tile t, stride 6): fuse_box = relu(1x1conv(y)+b) via one
    K=128 matmul (bias via ones partition); the ReLU evacuation also emits
    per-partition row sums (accum_out) for the global average pool.
    Edge tiles use weight blocks with zeroed rows/bias-columns so the
    pipeline stays uniform.
  SE chain: row sums -> gap -> 2-layer MLP -> sigmoid (replicated across
    partitions via weight replication) -> data-dependent boundary lhsT.
  phase B (per 6-row strip s): conv3x3 over concat(x, fuse_box) as 6
    accumulating matmuls (row-Toeplitz lhsT, K=128, dx via shifted rhs views;
    SAME padding realized with partial-column matmuls + has_written),
    mask/boundary heads as matmuls producing softmax channel-diff logits,
    sigmoids on ScalarE, add/clip on VectorE, final 1->16 expansion as a
    matmul with bias folded via ones partition, bf16 output DMA.
"""

import numpy as np
import ml_dtypes

BF16 = ml_dtypes.bfloat16

H = 512
W = 512
SB = 6                     # output rows per strip
NT = (H + SB - 1) // SB    # 86 strips/tiles
LAG = 12                   # strips between conv front and se-dependent tail
NPIX = float(H * W)
NW96 = 15                  # 96-col weight blocks
WCW = 96 * NW96 + 128 * 3  # wconst width

_cache = {}


# ----------------------------------------------------------------------------
# host-side weight layout builders
# ----------------------------------------------------------------------------

def _conv_lhsT(fc_w, half, zero_rows_from=None):
    """[3][128, 96]: W[dx][r*16+c, i*16+oc] = fc_w[oc, half*16+c, r-i, dx]
    for r-i in {0,1,2} (r: input row 0..7 within tile, i: output row 0..5)."""
    out = np.zeros((3, 128, 96), np.float32)
    for dx in range(3):
        for i in range(SB):
            for ky in range(3):
                r = i + ky
                out[dx, r * 16:r * 16 + 16, i * 16:i * 16 + 16] = \
                    fc_w[:, half * 16:half * 16 + 16, ky, dx].T
    if zero_rows_from is not None:
        out[:, zero_rows_from:, :] = 0.0
    return out


def _fuse_lhsT(fuse_w, fuse_b, zero_in_rows=(), zero_out_cols=()):
    """[128, 128]: rows (r*5+yc), r<8 -> 1x1 weights; row 40 (ones) -> bias."""
    out = np.zeros((128, 128), np.float32)
    for r in range(8):
        out[r * 5:r * 5 + 5, r * 16:r * 16 + 16] = fuse_w[:, :, 0, 0].T
        out[40, r * 16:r * 16 + 16] = fuse_b
    for a, b in zero_in_rows:
        out[a:b, :] = 0.0
    for a, b in zero_out_cols:
        out[:, a:b] = 0.0
    return out


def _mask_lhsT(fm_w, fm_b):
    out = np.zeros((128, 96), np.float32)
    d = fm_w[1, :, 0, 0] - fm_w[0, :, 0, 0]
    for i in range(SB):
        out[i * 16:i * 16 + 16, i] = d
    out[96, :SB] = fm_b[1] - fm_b[0]
    return out


def _cv_lhsT(cv_w, cv_b):
    out = np.zeros((128, 96), np.float32)
    for i in range(SB):
        out[i, i * 16:i * 16 + 16] = cv_w[:, 0, 0, 0]
        out[6, i * 16:i * 16 + 16] = cv_b
    return out


def _sel_lhsT():
    out = np.zeros((128, 96), np.float32)
    for r in range(1, 7):
        for fc in range(16):
            out[r * 16 + fc, fc] = 1.0 / NPIX
    return out


def _w1_lhsT(se_w1):
    out = np.zeros((128, 96), np.float32)
    out[:16, :16] = se_w1.T
    return out


def _w2rep_lhsT(se_w2):
    out = np.zeros((128, 96), np.float32)
    for r in range(SB):
        out[:16, r * 5:r * 5 + 5] = se_w2.T
    return out


def _p_lhsT(bd_w, bd_b):
    out = np.zeros((128, 96), np.float32)
    d = bd_w[1, :, 0, 0] - bd_w[0, :, 0, 0]
    for r in range(SB):
        out[r * 5:r * 5 + 5, r] = d
    out[30, :SB] = bd_b[1] - bd_b[0]
    return out


def _pack_weights(fuse_w, fuse_b, se_w1, se_w2, bd_w, bd_b, fc_w,
                  fm_w, fm_b, cv_w, cv_b):
    """[128, WCW]: 15 x 96-col blocks, then LF / LF_FIRST / LF_LAST (128 each).

    block order: wx0 wx1 wx2 wf0 wf1 wf2 lm lc sel w1l w2rep psb
                 wxl0 wxl1 wxl2 | LF LF_F LF_L
    """
    wx = _conv_lhsT(fc_w, 0)
    wf = _conv_lhsT(fc_w, 1)
    wxl = _conv_lhsT(fc_w, 0, zero_rows_from=48)
    blocks96 = [wx[0], wx[1], wx[2], wf[0], wf[1], wf[2],
                _mask_lhsT(fm_w, fm_b), _cv_lhsT(cv_w, cv_b), _sel_lhsT(),
                _w1_lhsT(se_w1), _w2rep_lhsT(se_w2), _p_lhsT(bd_w, bd_b),
                wxl[0], wxl[1], wxl[2]]
    out = np.zeros((128, WCW), np.float32)
    for i, b in enumerate(blocks96):
        out[:, i * 96:(i + 1) * 96] = b
    base = 96 * NW96
    out[:, base:base + 128] = _fuse_lhsT(fuse_w, fuse_b)
    # first tile: image row -1 invalid -> zero its y rows and its bias cols
    out[:, base + 128:base + 256] = _fuse_lhsT(
        fuse_w, fuse_b, zero_in_rows=[(0, 5)], zero_out_cols=[(0, 16)])
    # last tile: image rows 512.. invalid (tile rows 3..7)
    out[:, base + 256:base + 384] = _fuse_lhsT(
        fuse_w, fuse_b, zero_in_rows=[(15, 40)], zero_out_cols=[(48, 128)])
    return out.astype(BF16)


def _fcb_col(fc_b):
    """[128, 1] per-partition conv bias: partition i*16+oc -> fc_b[oc]."""
    out = np.zeros((128, 1), np.float32)
    for i in range(SB):
        out[i * 16:(i + 1) * 16, 0] = fc_b
    return out


# ----------------------------------------------------------------------------
# bass graph
# ----------------------------------------------------------------------------

def _build():
    import concourse.bass as bass
    import concourse.bacc as bacc
    import concourse.tile as tile
    from concourse import mybir

    f32 = mybir.dt.float32
    bf16 = mybir.dt.bfloat16
    AF = mybir.ActivationFunctionType
    ALU = mybir.AluOpType

    nc = bacc.Bacc("TRN2", target_bir_lowering=False)
    x_ext = nc.declare_dram_parameter("x", [16, H, W], bf16, isOutput=False)
    y_ext = nc.declare_dram_parameter("y", [5, H, W], bf16, isOutput=False)
    wc_ext = nc.declare_dram_parameter("wconst", [128, WCW], bf16,
                                       isOutput=False)
    fcb_ext = nc.declare_dram_parameter("fcb", [128, 1], f32, isOutput=False)
    cz_ext = nc.declare_dram_parameter("cz", [128, W], bf16, isOutput=False)
    co_ext = nc.declare_dram_parameter("co", [W], bf16, isOutput=False)
    cof_ext = nc.declare_dram_parameter("cof", [1], f32, isOutput=False)
    out_ext = nc.declare_dram_parameter("out", [16, H, W], bf16, isOutput=True)

    with tile.TileContext(nc) as tc:
        with (
            tc.tile_pool(name="singles", bufs=1) as singles,
            tc.tile_pool(name="sigring", bufs=3) as sigring,
            tc.tile_pool(name="outring", bufs=4) as outring,
            tc.tile_pool(name="ps_main", bufs=2, space="PSUM") as ps_main,
            tc.tile_pool(name="ps_mb", bufs=1, space="PSUM") as ps_mb,
        ):
            # ================= startup: constants + ring presets ============
            wc = singles.tile([128, WCW], bf16, tag="wc")
            nc.sync.dma_start(out=wc[:, :], in_=wc_ext[:, :])
            fcb = singles.tile([128, 1], f32, tag="fcb")
            nc.sync.dma_start(out=fcb[:, :], in_=fcb_ext[:, :])

            def wblk(i):
                return wc[:, i * 96:(i + 1) * 96]

            WX = [wblk(0), wblk(1), wblk(2)]
            WF = [wblk(3), wblk(4), wblk(5)]
            LM, LC, SEL, W1L, W2R, PSB = (wblk(6), wblk(7), wblk(8),
                                          wblk(9), wblk(10), wblk(11))
            WXL = [wblk(12), wblk(13), wblk(14)]
            base = 96 * NW96
            LF = wc[:, base:base + 128]
            LF_F = wc[:, base + 128:base + 256]
            LF_L = wc[:, base + 256:base + 384]

            # persistent fuse tiles + row-sum accumulator
            F = [singles.tile([128, W], bf16, tag=f"F{t}", name=f"F{t}")
                 for t in range(NT)]
            R = singles.tile([128, NT], f32, tag="R")
            nc.vector.memset(R[:, :], 0.0)

            # static rings with preset partitions
            NFCC = LAG + 4
            fcc = [singles.tile([128, W], bf16, tag=f"fcc{k}", name=f"fcc{k}")
                   for k in range(NFCC)]
            for k in range(NFCC):
                nc.sync.dma_start(out=fcc[k][96:97, :], in_=co_ext[:])
                nc.sync.dma_start(out=fcc[k][97:128, :], in_=cz_ext[0:31, :])
            NSV = 3
            sv = [singles.tile([128, W], bf16, tag=f"sv{k}", name=f"sv{k}")
                  for k in range(NSV)]
            for k in range(NSV):
                nc.sync.dma_start(out=sv[k][6:7, :], in_=co_ext[:])
                nc.sync.dma_start(out=sv[k][7:128, :], in_=cz_ext[0:121, :])
            xr = [singles.tile([128, W], bf16, tag=f"xr{k}", name=f"xr{k}")
                  for k in range(4)]
            nc.vector.memset(xr[0][0:16, :], 0.0)   # image row -1 of strip 0
            yhr = [singles.tile([128, W], bf16, tag=f"yhr{k}", name=f"yhr{k}")
                   for k in range(4)]
            for k in range(4):
                nc.sync.dma_start(out=yhr[k][40:41, :], in_=co_ext[:])
                nc.sync.dma_start(out=yhr[k][41:128, :], in_=cz_ext[0:87, :])
            nc.vector.memset(yhr[0][0:5, :], 0.0)   # image row -1 of tile 0
            ycr = [singles.tile([128, W], bf16, tag=f"ycr{k}", name=f"ycr{k}")
                   for k in range(4)]
            for k in range(4):
                nc.sync.dma_start(out=ycr[k][30:31, :], in_=co_ext[:])
                nc.sync.dma_start(out=ycr[k][31:128, :], in_=cz_ext[0:97, :])

            tc.strict_bb_all_engine_barrier()

            # ================= phase A: fuse tiles + row sums ===============
            for t in range(NT):
                r0 = SB * t - 1          # first input image row of tile t
                yh = yhr[t % 4]
                lo = max(0, -r0)
                hi = min(8, H - r0)
                nc.sync.dma_start(
                    out=yh[5 * lo:5 * hi, :],
                    in_=y_ext[:, r0 + lo:r0 + hi, :].rearrange("c r j -> r c j"),
                )
                lf = LF_F if t == 0 else (LF_L if hi < 8 else LF)
                fps = ps_main.tile([128, W], f32, tag="fuse")
                nc.tensor.matmul(fps[:, :], lhsT=lf, rhs=yh[:, :],
                                 start=True, stop=True)
                if t % 2 == 0:
                    nc.scalar.activation(out=F[t][:, :], in_=fps[:, :],
                                         func=AF.Relu,
                                         accum_out=R[:, t:t + 1])
                else:
                    nc.vector.tensor_scalar(out=F[t][:, :], in0=fps[:, :],
                                            scalar1=0.0, scalar2=0.0,
                                            op0=ALU.max, op1=ALU.add,
                                            accum_out=R[:, t:t + 1])

            # ================= SE chain =====================================
            R_bf = singles.tile([128, NT], bf16, tag="Rbf")
            nc.vector.tensor_copy(out=R_bf[:, :], in_=R[:, :])
            gps = ps_main.tile([96, NT], f32, tag="fuse")
            nc.tensor.matmul(gps[:, :], lhsT=SEL, rhs=R_bf[:, :],
                             start=True, stop=True)
            gap_f = singles.tile([96, 1], f32, tag="gapf")
            nc.vector.reduce_sum(out=gap_f[:, :], in_=gps[:, :],
                                 axis=mybir.AxisListType.X)
            gap_bf = singles.tile([128, 1], bf16, tag="gap")
            nc.vector.memset(gap_bf[:, :], 0.0)
            nc.vector.tensor_copy(out=gap_bf[0:96, :], in_=gap_f[:, :])
            hps = ps_main.tile([96, 1], f32, tag="fuse")
            nc.tensor.matmul(hps[:, :], lhsT=W1L, rhs=gap_bf[:, :],
                             start=True, stop=True)
            h_bf = singles.tile([128, 1], bf16, tag="hbf")
            nc.vector.memset(h_bf[:, :], 0.0)
            nc.scalar.activation(out=h_bf[0:96, :], in_=hps[:, :], func=AF.Relu)
            sps = ps_main.tile([96, 1], f32, tag="fuse")
            nc.tensor.matmul(sps[:, :], lhsT=W2R, rhs=h_bf[:, :],
                             start=True, stop=True)
            se_bc = singles.tile([128, 1], f32, tag="sebc")
            nc.vector.memset(se_bc[96:128, :], 0.0)
            nc.scalar.activation(out=se_bc[0:96, :], in_=sps[:, :],
                                 func=AF.Sigmoid)
            nc.sync.dma_start(out=se_bc[30:31, 0:1], in_=cof_ext[0:1])
            LB = singles.tile([128, 96], bf16, tag="lb")
            nc.vector.tensor_scalar_mul(out=LB[:, :], in0=PSB,
                                        scalar1=se_bc[:, :])

            # ================= phase B ======================================
            def issue_front(s):
                r0 = SB * s - 1
                lo = max(0, -r0)
                hi = min(8, H - r0)
                xt = xr[s % 4]
                nc.sync.dma_start(
                    out=xt[16 * lo:16 * hi, :],
                    in_=x_ext[:, r0 + lo:r0 + hi, :].rearrange("c r j -> r c j"),
                )
                wxs = WXL if hi < 8 else WX
                cps = ps_main.tile([96, W], f32, tag="conv")
                # center dx first: covers all 512 cols with start=True, so the
                # partial-column edge matmuls accumulate via has_written.
                nc.tensor.matmul(cps[:, 0:W], lhsT=wxs[1], rhs=xt[:, 0:W],
                                 start=True, stop=False)
                nc.tensor.matmul(cps[:, 0:W], lhsT=WF[1], rhs=F[s][:, 0:W],
                                 start=False, stop=False)
                nc.tensor.matmul(cps[:, 1:W], lhsT=wxs[0], rhs=xt[:, 0:W - 1],
                                 start=False, stop=False)
                nc.tensor.matmul(cps[:, 1:W], lhsT=WF[0], rhs=F[s][:, 0:W - 1],
                                 start=False, stop=False)
                nc.tensor.matmul(cps[:, 0:W - 1], lhsT=wxs[2], rhs=xt[:, 1:W],
                                 start=False, stop=False)
                nc.tensor.matmul(cps[:, 0:W - 1], lhsT=WF[2], rhs=F[s][:, 1:W],
                                 start=False, stop=True)
                fc = fcc[s % NFCC]
                # fcc = relu(conv + fc_b)
                nc.vector.tensor_scalar(out=fc[0:96, :], in0=cps[:, :],
                                        scalar1=fcb[0:96, :], scalar2=0.0,
                                        op0=ALU.add, op1=ALU.max)

            def issue_tail(u):
                r0c = SB * u          # first output row of strip u
                nrow = min(SB, H - r0c)
                fc = fcc[u % NFCC]
                yc = ycr[u % 4]
                nc.sync.dma_start(
                    out=yc[0:5 * nrow, :],
                    in_=y_ext[:, r0c:r0c + nrow, :].rearrange("c r j -> r c j"),
                )
                mps = ps_mb.tile([96, W], f32, tag="mbm")
                nc.tensor.matmul(mps[:, :], lhsT=LM, rhs=fc[:, :],
                                 start=True, stop=True)
                bps = ps_mb.tile([96, W], f32, tag="mbb")
                nc.tensor.matmul(bps[:, :], lhsT=LB[:, :], rhs=yc[:, :],
                                 start=True, stop=True)
                sgm = sigring.tile([6, W], bf16, tag="sgm")
                nc.scalar.activation(out=sgm[:, :], in_=mps[0:6, :],
                                     func=AF.Sigmoid)
                sgb = sigring.tile([6, W], bf16, tag="sgb")
                nc.scalar.activation(out=sgb[:, :], in_=bps[0:6, :],
                                     func=AF.Sigmoid)
                svt = sv[u % NSV]
                nc.vector.tensor_add(out=svt[0:6, :], in0=sgm[:, :],
                                     in1=sgb[:, :])
                nc.vector.tensor_scalar_min(out=svt[0:6, :], in0=svt[0:6, :],
                                            scalar1=1.0)
                ops = ps_main.tile([96, W], f32, tag="cv")
                nc.tensor.matmul(ops[:, :], lhsT=LC, rhs=svt[:, :],
                                 start=True, stop=True)
                ot = outring.tile([96, W], bf16, tag="out")
                nc.scalar.copy(out=ot[:, :], in_=ops[:, :])
                nc.sync.dma_start(
                    out=out_ext[:, r0c:r0c + nrow, :].rearrange("c r j -> r c j"),
                    in_=ot[0:16 * nrow, :],
                )

            for s in range(NT + LAG):
                if s < NT:
                    issue_front(s)
                u = s - LAG
                if u >= 0:
                    issue_tail(u)

    nc.compile()
    return nc


# ----------------------------------------------------------------------------
# entry point
# ----------------------------------------------------------------------------

LAST_RESULT = None


def prepare(x, y, fuse_w, fuse_b, se_w1, se_w2, bd_w, bd_b,
            fc_w, fc_b, fm_w, fm_b, cv_w, cv_b):
    """Build (cached) graph + per-core input maps."""
    if "nc" not in _cache:
        _cache["nc"] = _build()
    nc = _cache["nc"]

    wconst = _pack_weights(np.asarray(fuse_w, np.float32),
                           np.asarray(fuse_b, np.float32),
                           np.asarray(se_w1, np.float32),
                           np.asarray(se_w2, np.float32),
                           np.asarray(bd_w, np.float32),
                           np.asarray(bd_b, np.float32),
                           np.asarray(fc_w, np.float32),
                           np.asarray(fm_w, np.float32),
                           np.asarray(fm_b, np.float32),
                           np.asarray(cv_w, np.float32),
                           np.asarray(cv_b, np.float32))
    fcb = _fcb_col(np.asarray(fc_b, np.float32))
    xb = np.asarray(x, np.float32).astype(BF16)
    yb = np.asarray(y, np.float32).astype(BF16)

    cz = np.zeros((128, W), BF16)
    co = np.ones((W,), BF16)
    cof = np.ones((1,), np.float32)
    in_maps = [
        {"x": np.ascontiguousarray(xb[i]), "y": np.ascontiguousarray(yb[i]),
         "wconst": wconst, "fcb": fcb, "cz": cz, "co": co, "cof": cof}
        for i in range(8)
    ]
    return nc, in_maps


def kernel(x, y, fuse_w, fuse_b, se_w1, se_w2, bd_w, bd_b,
           fc_w, fc_b, fm_w, fm_b, cv_w, cv_b):
    global LAST_RESULT
    from concourse.bass_utils import run_bass_kernel_spmd

    nc, in_maps = prepare(x, y, fuse_w, fuse_b, se_w1, se_w2, bd_w, bd_b,
                          fc_w, fc_b, fm_w, fm_b, cv_w, cv_b)
    res = run_bass_kernel_spmd(nc, in_maps, core_ids=list(range(8)))
    LAST_RESULT = res
    out = np.stack([np.asarray(res.results[i]["out"], np.float32)
                    for i in range(8)])
    return out
